# revision 10
# baseline (speedup 1.0000x reference)
"""Trainium2 Bass kernel for nn_KCN_38955353375381 (dense_mlp).

Reference computation (per token n, D=512, K=8 shifts, P=8 petals):
  phi[n, d*8+k] = softplus(x[n,d] + s_k)                  s = linspace(-1,1,8)
  x_proj = phi @ (softplus(phi_raw)**2).T + phi_bias      [N, 512]
  z0     = softplus(x_proj * sigmoid(gate_raw[p]))        (shared: gates equal)
  z1     = softplus((z0 @ sp(raw_weight2[p]).T**2 + bias2[p]) * sigmoid(gate_raw2[p]))
  x_res  = x @ (z_weight[p,:512] + z_weight[p,512:])
  out[n,p,:] = softplus(z1 + x_res) + output_bias[p]

Two SPMD dispatches on 8 NeuronCores:

1. prep: the parameter transforms (softplus^2, PE transposes into matmul
   weight layout, z_weight half-fold, bf16 cast) sharded across the 8 cores
   (each core preps 1/8 of the parameter bytes).  The host only slices /
   reassembles arrays between dispatches.
2. main: pure data parallel over the 4096 tokens -> 512 tokens per core;
   transformed weights replicated.

softplus is computed as Ln(Exp(x)*scale + 1) on the ACT engine (the deployed
walrus activation tables have no softplus set; exp and ln share one set).
Matmuls run in bf16 (inputs rounded to bf16, fp32 PSUM accumulation).
"""

import contextlib
import sys

for _p in ("/opt/trn_rl_repo",):
    if _p not in sys.path:
        sys.path.insert(0, _p)

import ml_dtypes
import numpy as np


def _force_single_act_set():
    """Point walrus at an act-table root containing only the
    natural_log_exp_and_others set (exp + ln).  With the full table the
    set-selection pass alternates sets between Exp and Ln activations,
    inserting a ~1.3us ACT_TABLE_LOAD per switch (~48us per kernel run).
    All activations in these programs are exp/ln, so one set suffices."""
    import json
    import shutil
    import tempfile

    if os.environ.get("BASS_ACT_ROOT_JSON_PATH"):
        return
    try:
        import neuronxcc

        pwp = os.path.join(os.path.dirname(neuronxcc.__file__), "pwp",
                           "pwp_bin_trainium")
        info = json.load(open(os.path.join(pwp, "act_info.json")))
        keep = [s for s in info["act_func_sets"]
                if s["name"] == "natural_log_exp_and_others"]
        if not keep:
            return
        tmpd = tempfile.mkdtemp(prefix="act_root_")
        files = [keep[0]["bkt_bin"], keep[0]["ctrl_bin"], keep[0]["profile_json"]]
        for f in files:
            shutil.copy(os.path.join(pwp, f), os.path.join(tmpd, f))
        out = dict(info)
        out["act_func_sets"] = keep
        with open(os.path.join(tmpd, "act_info.json"), "w") as fh:
            json.dump(out, fh)
        os.environ["BASS_ACT_ROOT_JSON_PATH"] = os.path.join(tmpd, "act_info.json")
    except Exception:
        pass  # fall back to the default tables (slower, still correct)


import os

_force_single_act_set()

import concourse.bacc as bacc
import concourse.mybir as mybir
import concourse.tile as tile
from concourse import masks
from concourse.bass_utils import run_bass_kernel_spmd

if os.environ.get("BASS_ACT_ROOT_JSON_PATH"):
    # Keep bass's pre-placed InstLoadActFuncSet ids consistent with the
    # single-set act root installed above: both sides must see set 0 ==
    # natural_log_exp_and_others.
    import concourse.hw_specs as _hw_specs

    _orig_get_act_tables = _hw_specs.get_activation_tables

    def _single_set_act_tables(module_arch):
        t = _orig_get_act_tables(module_arch)
        return {"natural_log_exp_and_others": t["natural_log_exp_and_others"]}

    _hw_specs.get_activation_tables = _single_set_act_tables
    bacc.get_activation_tables = _single_set_act_tables

F32 = mybir.dt.float32
BF16 = mybir.dt.bfloat16
AF = mybir.ActivationFunctionType
NPBF16 = ml_dtypes.bfloat16

D = 512          # feature dim (D_IN == D_OUT)
K = 8            # shifts
P = 8            # petals
N_CORES = 8
NT = 512         # tokens per core
NC = 4           # 128-token chunks per core
DC = 4           # 128-feature chunks

_CACHE = {}
_RUN_KWARGS = {}


def _softplus_sq_bf16(nc, pool, tc, src_ap, name):
    """softplus(x)^2 on a [128, 2048] f32 tile (in place), squared+cast bf16."""
    nc.scalar.activation(src_ap, src_ap, AF.Exp)
    nc.scalar.activation(src_ap, src_ap, AF.Ln, bias=1.0)
    q = pool.tile([128, 4 * D], BF16, tag=name, name=name)
    nc.vector.tensor_mul(q[:], src_ap, src_ap)
    return q


def _build_prep():
    """Per-core parameter prep.  Inputs are host-sliced so the same program
    does 1/8 of the work on each core:
      phr_part [128, 2048] f32  rows ec=c//2 of phi_raw, cols (kk, d) for
                                k-half c%2 (kk = k%4)
      w2_part  [128, 2048] f32  [e_loc, er*512+d] = raw_weight2[c, er*128+e_loc, d]
      zw_part  [1024, 512] f32  z_weight[c]
    Outputs (bf16):
      wphiT_part [128, 2048]    block (kk,dc) at free (kk*4+dc)*128 holds
                                transpose(sp(phr block)^2)  -> [d_loc, e_loc]
      w2T_part   [128, 2048]    [d_loc, dc*512+e] = sp(raw_weight2[c,e,dc*128+d_loc])^2
      zws_part   [128, 2048]    [d_loc, dc*512+e] = zw[c,dc*128+d_loc,e]+zw[c,512+dc*128+d_loc,e]
    """
    nc = bacc.Bacc("TRN2", target_bir_lowering=False, debug=False)
    phr_d = nc.dram_tensor("phr_part", [128, 4 * D], F32, kind="ExternalInput").ap()
    w2_d = nc.dram_tensor("w2_part", [128, 4 * D], F32, kind="ExternalInput").ap()
    zw_d = nc.dram_tensor("zw_part", [2 * D, D], F32, kind="ExternalInput").ap()
    wphiT_d = nc.dram_tensor("wphiT_part", [128, 4 * D], BF16,
                             kind="ExternalOutput").ap()
    w2T_d = nc.dram_tensor("w2T_part", [128, 4 * D], BF16,
                           kind="ExternalOutput").ap()
    zws_d = nc.dram_tensor("zws_part", [128, 4 * D], BF16,
                           kind="ExternalOutput").ap()

    with tile.TileContext(nc) as tc, contextlib.ExitStack() as ctx:
        cpool = ctx.enter_context(tc.tile_pool(name="consts", bufs=1))
        id16 = cpool.tile([128, 128], BF16)
        masks.make_identity(nc, id16[:])

        ps_tr = ctx.enter_context(tc.tile_pool(name="ps_tr", bufs=2, space="PSUM"))
        pool = ctx.enter_context(tc.tile_pool(name="p", bufs=1))

        pa = pool.tile([128, 4 * D], F32, tag="pa")
        nc.sync.dma_start(pa[:], phr_d[:])
        paq = _softplus_sq_bf16(nc, pool, tc, pa[:], "paq")
        wout = pool.tile([128, 4 * D], BF16, tag="wout")
        for g in range(4):  # 4 blocks of 4 transposes -> one psum tile each
            pt = ps_tr.tile([128, 512], BF16, tag="tr", name=f"pt{g}")
            for b in range(4):
                i = g * 4 + b
                nc.tensor.transpose(
                    pt[:, b * 128 : (b + 1) * 128],
                    paq[:, i * 128 : (i + 1) * 128],
                    id16[:],
                )
            nc.vector.tensor_copy(wout[:, g * 512 : (g + 1) * 512], pt[:])
        nc.sync.dma_start(wphiT_d[:], wout[:])

        pb = pool.tile([128, 4 * D], F32, tag="pb")
        nc.sync.dma_start(pb[:], w2_d[:])
        pbq = _softplus_sq_bf16(nc, pool, tc, pb[:], "pbq")
        w2out = pool.tile([128, 4 * D], BF16, tag="w2out")
        for dc in range(DC):
            pt = ps_tr.tile([128, 512], BF16, tag="tr", name=f"qt{dc}")
            for er in range(4):
                nc.tensor.transpose(
                    pt[:, er * 128 : (er + 1) * 128],
                    pbq[:, er * D + dc * 128 : er * D + (dc + 1) * 128],
                    id16[:],
                )
            nc.vector.tensor_copy(w2out[:, dc * D : (dc + 1) * D], pt[:])
        nc.sync.dma_start(w2T_d[:], w2out[:])

        zs = pool.tile([128, 4 * D], F32, tag="zs")
        for dc in range(DC):
            nc.sync.dma_start(
                zs[:, dc * D : (dc + 1) * D], zw_d[dc * 128 : (dc + 1) * 128, :]
            )
            nc.gpsimd.dma_start(
                zs[:, dc * D : (dc + 1) * D],
                zw_d[D + dc * 128 : D + (dc + 1) * 128, :],
                accum_op=mybir.AluOpType.add,
            )
        zsb = pool.tile([128, 4 * D], BF16, tag="zsb")
        nc.vector.tensor_copy(zsb[:], zs[:])
        nc.sync.dma_start(zws_d[:], zsb[:])

    nc.compile()
    return nc


def _build_main(g1s, g2, ck):
    """Per-core main program (zero biases, shared g1 gate)."""
    nc = bacc.Bacc("TRN2", target_bir_lowering=False, debug=False)

    x_d = nc.dram_tensor("x", [NT, D], F32, kind="ExternalInput").ap()
    wphiT_d = nc.dram_tensor("wphiT", [DC, 128, K * D], BF16,
                             kind="ExternalInput").ap()
    w2T_d = nc.dram_tensor("w2T", [P, 128, 4 * D], BF16,
                           kind="ExternalInput").ap()
    zws_d = nc.dram_tensor("zws", [P, 128, 4 * D], BF16,
                           kind="ExternalInput").ap()
    out_d = nc.dram_tensor("out", [NT, P, D], F32, kind="ExternalOutput").ap()
    out_r = out_d.rearrange("(a b) p e -> b a p e", b=128)

    with tile.TileContext(nc) as tc, contextlib.ExitStack() as ctx:
        const_pool = ctx.enter_context(tc.tile_pool(name="consts", bufs=1))
        id32 = const_pool.tile([128, 128], F32)
        masks.make_identity(nc, id32[:])

        persist = ctx.enter_context(tc.tile_pool(name="persist", bufs=1))
        xE = persist.tile([128, DC * NT], F32, tag="xE")      # x^T, then exp(x^T)
        xp_sb = persist.tile([128, DC * NT], F32, tag="xp")   # x_proj^T, then exp
        z0T = persist.tile([128, DC * NT], BF16, tag="z0T")
        # x_res for every petal, [n_loc, (p, jp, js, e)] f32
        xres = persist.tile([128, P * 4 * D], F32, tag="xres")

        with contextlib.ExitStack() as s1:
            ps_tr = s1.enter_context(
                tc.tile_pool(name="ps_tr", bufs=2, space="PSUM"))
            ps_mm1 = s1.enter_context(
                tc.tile_pool(name="ps_mm1", bufs=2, space="PSUM"))
            ps_r = s1.enter_context(
                tc.tile_pool(name="ps_r", bufs=2, space="PSUM"))
            stage1 = s1.enter_context(tc.tile_pool(name="stage1", bufs=1))
            xTb = stage1.tile([128, DC * NT], BF16, tag="xTb")
            phi = [
                stage1.tile([128, DC * NT], BF16, tag=f"phi{k}", name=f"phi{k}")
                for k in range(K)
            ]

            # ---------------- stage A: load x, transpose, exp, phi ------------
            xa_pool = s1.enter_context(tc.tile_pool(name="xa", bufs=4))
            xn = []
            for j in range(NC):
                t = xa_pool.tile([128, D], F32, tag="xn", name=f"xn{j}")
                nc.sync.dma_start(t[:], x_d[j * 128 : (j + 1) * 128, :])
                xn.append(t)
            for dc in range(DC):
                pt = ps_tr.tile([128, NT], F32, tag="tr", name=f"xtp{dc}")
                for j in range(NC):
                    nc.tensor.transpose(
                        pt[:, j * 128 : (j + 1) * 128],
                        xn[j][:, dc * 128 : (dc + 1) * 128],
                        id32[:],
                    )
                nc.vector.tensor_copy(xE[:, dc * NT : (dc + 1) * NT], pt[:])
                nc.vector.tensor_copy(xTb[:, dc * NT : (dc + 1) * NT], pt[:])

            nc.scalar.activation(xE[:], xE[:], AF.Exp)  # in place: exp(x^T)
            for k in range(K):
                nc.scalar.activation(
                    phi[k][:], xE[:], AF.Ln, bias=1.0, scale=float(ck[k])
                )

            # ------------- stage A2: x_res for all petals (PE warmup) ---------
            pet_pool = s1.enter_context(tc.tile_pool(name="pet", bufs=3))
            for p in range(P):
                zsb = pet_pool.tile([128, 4 * D], BF16, tag="zsb", name=f"zsb{p}")
                nc.sync.dma_start(zsb[:], zws_d[p])
                for jp in range(2):
                    psR = ps_r.tile([128, 1024], F32, tag="r", name=f"psR{p}_{jp}")
                    for js in range(2):
                        j = jp * 2 + js
                        for dc in range(DC):
                            nc.tensor.matmul(
                                psR[:, js * D : (js + 1) * D],
                                xTb[:, dc * NT + j * 128 : dc * NT + (j + 1) * 128],
                                zsb[:, dc * D : (dc + 1) * D],
                                start=(dc == 0),
                                stop=(dc == DC - 1),
                            )
                    off = p * 4 * D + jp * 1024
                    nc.vector.tensor_copy(xres[:, off : off + 1024], psR[:])

            # ---------------- stage B: mm1 (x_proj^T per e-chunk) -------------
            wphi_pool = s1.enter_context(tc.tile_pool(name="wphi", bufs=2))
            for ec in range(DC):
                wsb = wphi_pool.tile([128, K * D], BF16, tag="wsb", name=f"wsb{ec}")
                nc.sync.dma_start(wsb[:], wphiT_d[ec])
                xp_ps = ps_mm1.tile([128, NT], F32, tag="mm1", name=f"xp_ps{ec}")
                first = True
                for k in range(K):
                    for dc in range(DC):
                        off = (k // 4) * 2048 + (k % 4) * 512 + dc * 128
                        last = (k == K - 1) and (dc == DC - 1)
                        nc.tensor.matmul(
                            xp_ps[:],
                            wsb[:, off : off + 128],
                            phi[k][:, dc * NT : (dc + 1) * NT],
                            start=first,
                            stop=last,
                        )
                        first = False
                nc.vector.tensor_copy(xp_sb[:, ec * NT : (ec + 1) * NT], xp_ps[:])

        # ---------------- stage C: shared z0 ----------------
        nc.scalar.activation(xp_sb[:], xp_sb[:], AF.Exp, scale=float(g1s))
        nc.scalar.activation(z0T[:], xp_sb[:], AF.Ln, bias=1.0)

        # ---------------- stage D: petals ----------------
        ps_pet = ctx.enter_context(tc.tile_pool(name="ps_pet", bufs=4,
                                                space="PSUM"))
        pet2_pool = ctx.enter_context(tc.tile_pool(name="pet2", bufs=2))
        zf_pool = ctx.enter_context(tc.tile_pool(name="zf", bufs=3))

        for p in range(P):
            w2sb = pet2_pool.tile([128, 4 * D], BF16, tag="w2sb", name=f"w2sb{p}")
            nc.sync.dma_start(w2sb[:], w2T_d[p])

            g2p = float(g2[p])
            for jp in range(2):  # pairs of 128-token chunks
                psA = ps_pet.tile([128, 1024], F32, tag="pet", name=f"psA{p}_{jp}")
                for js in range(2):
                    j = jp * 2 + js
                    for dc in range(DC):
                        nc.tensor.matmul(
                            psA[:, js * D : (js + 1) * D],
                            z0T[:, dc * NT + j * 128 : dc * NT + (j + 1) * 128],
                            w2sb[:, dc * D : (dc + 1) * D],
                            start=(dc == 0),
                            stop=(dc == DC - 1),
                        )
                # z_final = ln(1 + (1 + e^{g2 u}) e^r)  [u = z0@w2T, r = x@zws]
                # == softplus(softplus(g2 u) + r); one ACT pass fewer.
                t1 = zf_pool.tile([128, 1024], F32, tag="t1", name=f"t1_{p}{jp}")
                nc.scalar.activation(t1[:], psA[:], AF.Exp, scale=g2p)
                t2 = zf_pool.tile([128, 1024], F32, tag="t2", name=f"t2_{p}{jp}")
                off = p * 4 * D + jp * 1024
                nc.scalar.activation(t2[:], xres[:, off : off + 1024], AF.Exp)
                nc.vector.tensor_mul(t1[:], t1[:], t2[:])
                nc.vector.tensor_add(t1[:], t1[:], t2[:])
                zf = zf_pool.tile([128, 1024], F32, tag="zfo", name=f"zf{p}_{jp}")
                nc.scalar.activation(zf[:], t1[:], AF.Ln, bias=1.0)

                zf_r = zf[:].rearrange("n (js e) -> n js e", js=2)
                nc.sync.dma_start(out_r[:, jp * 2 : (jp + 1) * 2, p, :], zf_r)

    nc.compile()
    return nc


def _prep_scalars(inputs):
    gate_raw = np.asarray(inputs["gate_raw"], dtype=np.float32)
    gate_raw2 = np.asarray(inputs["gate_raw2"], dtype=np.float32)
    g1 = 1.0 / (1.0 + np.exp(-gate_raw.astype(np.float64)))
    g2 = 1.0 / (1.0 + np.exp(-gate_raw2.astype(np.float64)))
    shifts = np.linspace(-1.0, 1.0, K, dtype=np.float32)
    ck = np.exp(shifts.astype(np.float64))

    if not bool(np.all(gate_raw == gate_raw[0])):
        raise NotImplementedError("per-petal gate_raw values")
    for name in ("phi_bias", "bias2", "output_bias"):
        if bool(np.any(np.asarray(inputs[name]))):
            raise NotImplementedError(f"nonzero {name} not supported")
    return g1, g2, ck


def _get_programs(inputs):
    g1, g2, ck = _prep_scalars(inputs)
    key = (tuple(np.float32(g1)), tuple(np.float32(g2)), tuple(np.float32(ck)))
    if key not in _CACHE:
        _CACHE[key] = (_build_prep(), _build_main(g1[0], g2, ck))
    return _CACHE[key]


def kernel(**inputs):
    nc_prep, nc_main = _get_programs(inputs)

    x = np.ascontiguousarray(np.asarray(inputs["x"], dtype=np.float32))
    orig_shape = x.shape
    x_flat = x.reshape(-1, D)
    assert x_flat.shape[0] == N_CORES * NT

    phr = np.asarray(inputs["phi_raw"], dtype=np.float32)
    w2 = np.asarray(inputs["raw_weight2"], dtype=np.float32)
    zw = np.asarray(inputs["z_weight"], dtype=np.float32)

    # ---- dispatch 1: parameter prep, work sharded across cores ----
    prep_maps = []
    for c in range(N_CORES):
        ec, kh = c // 2, c % 2
        rows = phr[ec * 128 : (ec + 1) * 128].reshape(128, D, K)
        phr_part = np.ascontiguousarray(
            rows[:, :, kh * 4 : (kh + 1) * 4].transpose(0, 2, 1).reshape(128, 4 * D)
        )
        w2_part = np.ascontiguousarray(
            w2[c].reshape(4, 128, D).transpose(1, 0, 2).reshape(128, 4 * D)
        )
        prep_maps.append(
            {
                "phr_part": phr_part,
                "w2_part": w2_part,
                "zw_part": np.ascontiguousarray(zw[c]),
            }
        )
    res1 = run_bass_kernel_spmd(nc_prep, prep_maps, core_ids=list(range(N_CORES)),
                                **_RUN_KWARGS)

    wphiT = np.empty((DC, 128, K * D), dtype=NPBF16)
    w2T = np.empty((P, 128, 4 * D), dtype=NPBF16)
    zws = np.empty((P, 128, 4 * D), dtype=NPBF16)
    for c in range(N_CORES):
        r = res1.results[c]
        wphiT[c // 2, :, (c % 2) * 2048 : (c % 2 + 1) * 2048] = r["wphiT_part"]
        w2T[c] = r["w2T_part"]
        zws[c] = r["zws_part"]

    # ---- dispatch 2: main, data parallel over tokens ----
    main_maps = []
    for c in range(N_CORES):
        main_maps.append(
            {
                "x": np.ascontiguousarray(x_flat[c * NT : (c + 1) * NT]),
                "wphiT": wphiT,
                "w2T": w2T,
                "zws": zws,
            }
        )
    res2 = run_bass_kernel_spmd(nc_main, main_maps, core_ids=list(range(N_CORES)),
                                **_RUN_KWARGS)

    out = np.concatenate([res2.results[c]["out"] for c in range(N_CORES)], axis=0)
    kernel.last_results = (res1, res2)
    return out.reshape(tuple(orig_shape[:-1]) + (P, D))


kernel.last_results = None


# revision 13
# speedup vs baseline: 1.1058x; 1.1058x over previous
"""Trainium2 Bass kernel for nn_KCN_38955353375381 (dense_mlp).

Reference computation (per token n, D=512, K=8 shifts, P=8 petals):
  phi[n, d*8+k] = softplus(x[n,d] + s_k)                  s = linspace(-1,1,8)
  x_proj = phi @ (softplus(phi_raw)**2).T + phi_bias      [N, 512]
  z0     = softplus(x_proj * sigmoid(gate_raw[p]))        (shared: gates equal)
  z1     = softplus((z0 @ sp(raw_weight2[p]).T**2 + bias2[p]) * sigmoid(gate_raw2[p]))
  x_res  = x @ (z_weight[p,:512] + z_weight[p,512:])
  out[n,p,:] = softplus(z1 + x_res) + output_bias[p]

Two SPMD dispatches on 8 NeuronCores:

1. prep: the parameter transforms (softplus^2, PE transposes into matmul
   weight layout, z_weight half-fold, bf16 cast) sharded across the 8 cores
   (each core preps 1/8 of the parameter bytes).  The host only slices /
   reassembles arrays between dispatches.
2. main: pure data parallel over the 4096 tokens -> 512 tokens per core;
   transformed weights replicated.

softplus is computed as Ln(Exp(x)*scale + 1) on the ACT engine (the deployed
walrus activation tables have no softplus set; exp and ln share one set).
Matmuls run in bf16 (inputs rounded to bf16, fp32 PSUM accumulation).
"""

import contextlib
import sys

for _p in ("/opt/trn_rl_repo",):
    if _p not in sys.path:
        sys.path.insert(0, _p)

import ml_dtypes
import numpy as np


def _force_single_act_set():
    """Point walrus at an act-table root containing only the
    natural_log_exp_and_others set (exp + ln).  With the full table the
    set-selection pass alternates sets between Exp and Ln activations,
    inserting a ~1.3us ACT_TABLE_LOAD per switch (~48us per kernel run).
    All activations in these programs are exp/ln, so one set suffices."""
    import json
    import shutil
    import tempfile

    if os.environ.get("BASS_ACT_ROOT_JSON_PATH"):
        return
    try:
        import neuronxcc

        pwp = os.path.join(os.path.dirname(neuronxcc.__file__), "pwp",
                           "pwp_bin_trainium")
        info = json.load(open(os.path.join(pwp, "act_info.json")))
        keep = [s for s in info["act_func_sets"]
                if s["name"] == "natural_log_exp_and_others"]
        if not keep:
            return
        tmpd = tempfile.mkdtemp(prefix="act_root_")
        files = [keep[0]["bkt_bin"], keep[0]["ctrl_bin"], keep[0]["profile_json"]]
        for f in files:
            shutil.copy(os.path.join(pwp, f), os.path.join(tmpd, f))
        out = dict(info)
        out["act_func_sets"] = keep
        with open(os.path.join(tmpd, "act_info.json"), "w") as fh:
            json.dump(out, fh)
        os.environ["BASS_ACT_ROOT_JSON_PATH"] = os.path.join(tmpd, "act_info.json")
    except Exception:
        pass  # fall back to the default tables (slower, still correct)


import os

_force_single_act_set()

import concourse.bacc as bacc
import concourse.mybir as mybir
import concourse.tile as tile
from concourse import masks
from concourse.bass_utils import run_bass_kernel_spmd

if os.environ.get("BASS_ACT_ROOT_JSON_PATH"):
    # Keep bass's pre-placed InstLoadActFuncSet ids consistent with the
    # single-set act root installed above: both sides must see set 0 ==
    # natural_log_exp_and_others.
    import concourse.hw_specs as _hw_specs

    _orig_get_act_tables = _hw_specs.get_activation_tables

    def _single_set_act_tables(module_arch):
        t = _orig_get_act_tables(module_arch)
        return {"natural_log_exp_and_others": t["natural_log_exp_and_others"]}

    _hw_specs.get_activation_tables = _single_set_act_tables
    bacc.get_activation_tables = _single_set_act_tables

F32 = mybir.dt.float32
BF16 = mybir.dt.bfloat16
AF = mybir.ActivationFunctionType
NPBF16 = ml_dtypes.bfloat16

D = 512          # feature dim (D_IN == D_OUT)
K = 8            # shifts
P = 8            # petals
N_CORES = 8
NT = 512         # tokens per core
NC = 4           # 128-token chunks per core
DC = 4           # 128-feature chunks

_CACHE = {}
_RUN_KWARGS = {}


def _softplus_sq_bf16(nc, pool, tc, src_ap, name):
    """softplus(x)^2 on a [128, 2048] f32 tile (in place), squared+cast bf16."""
    nc.scalar.activation(src_ap, src_ap, AF.Exp)
    nc.scalar.activation(src_ap, src_ap, AF.Ln, bias=1.0)
    q = pool.tile([128, 4 * D], BF16, tag=name, name=name)
    nc.vector.tensor_mul(q[:], src_ap, src_ap)
    return q


def _build_prep():
    """Per-core parameter prep.  Inputs are host-sliced so the same program
    does 1/8 of the work on each core:
      phr_part [128, 2048] f32  rows ec=c//2 of phi_raw, cols (kk, d) for
                                k-half c%2 (kk = k%4)
      w2_part  [128, 2048] f32  [e_loc, er*512+d] = raw_weight2[c, er*128+e_loc, d]
      zw_part  [1024, 512] f32  z_weight[c]
    Outputs (bf16):
      wphiT_part [128, 2048]    block (kk,dc) at free (kk*4+dc)*128 holds
                                transpose(sp(phr block)^2)  -> [d_loc, e_loc]
      w2T_part   [128, 2048]    [d_loc, dc*512+e] = sp(raw_weight2[c,e,dc*128+d_loc])^2
      zws_part   [128, 2048]    [d_loc, dc*512+e] = zw[c,dc*128+d_loc,e]+zw[c,512+dc*128+d_loc,e]
    """
    nc = bacc.Bacc("TRN2", target_bir_lowering=False, debug=False)
    phr_d = nc.dram_tensor("phr_part", [128, 4 * D], F32, kind="ExternalInput").ap()
    w2_d = nc.dram_tensor("w2_part", [128, 4 * D], F32, kind="ExternalInput").ap()
    zw_d = nc.dram_tensor("zw_part", [2 * D, D], F32, kind="ExternalInput").ap()
    wphiT_d = nc.dram_tensor("wphiT_part", [128, 4 * D], BF16,
                             kind="ExternalOutput").ap()
    w2T_d = nc.dram_tensor("w2T_part", [128, 4 * D], BF16,
                           kind="ExternalOutput").ap()
    zws_d = nc.dram_tensor("zws_part", [128, 4 * D], BF16,
                           kind="ExternalOutput").ap()

    with tile.TileContext(nc) as tc, contextlib.ExitStack() as ctx:
        cpool = ctx.enter_context(tc.tile_pool(name="consts", bufs=1))
        id16 = cpool.tile([128, 128], BF16)
        masks.make_identity(nc, id16[:])

        ps_tr = ctx.enter_context(tc.tile_pool(name="ps_tr", bufs=2, space="PSUM"))
        pool = ctx.enter_context(tc.tile_pool(name="p", bufs=1))

        pa = pool.tile([128, 4 * D], F32, tag="pa")
        nc.sync.dma_start(pa[:], phr_d[:])
        paq = _softplus_sq_bf16(nc, pool, tc, pa[:], "paq")
        wout = pool.tile([128, 4 * D], BF16, tag="wout")
        for g in range(4):  # 4 blocks of 4 transposes -> one psum tile each
            pt = ps_tr.tile([128, 512], BF16, tag="tr", name=f"pt{g}")
            for b in range(4):
                i = g * 4 + b
                nc.tensor.transpose(
                    pt[:, b * 128 : (b + 1) * 128],
                    paq[:, i * 128 : (i + 1) * 128],
                    id16[:],
                )
            nc.vector.tensor_copy(wout[:, g * 512 : (g + 1) * 512], pt[:])
        nc.sync.dma_start(wphiT_d[:], wout[:])

        pb = pool.tile([128, 4 * D], F32, tag="pb")
        nc.sync.dma_start(pb[:], w2_d[:])
        pbq = _softplus_sq_bf16(nc, pool, tc, pb[:], "pbq")
        w2out = pool.tile([128, 4 * D], BF16, tag="w2out")
        for dc in range(DC):
            pt = ps_tr.tile([128, 512], BF16, tag="tr", name=f"qt{dc}")
            for er in range(4):
                nc.tensor.transpose(
                    pt[:, er * 128 : (er + 1) * 128],
                    pbq[:, er * D + dc * 128 : er * D + (dc + 1) * 128],
                    id16[:],
                )
            nc.vector.tensor_copy(w2out[:, dc * D : (dc + 1) * D], pt[:])
        nc.sync.dma_start(w2T_d[:], w2out[:])

        zs0 = pool.tile([128, 4 * D], F32, tag="zs0")
        zs1 = pool.tile([128, 4 * D], F32, tag="zs1")
        for dc in range(DC):
            nc.sync.dma_start(
                zs0[:, dc * D : (dc + 1) * D], zw_d[dc * 128 : (dc + 1) * 128, :]
            )
            nc.sync.dma_start(
                zs1[:, dc * D : (dc + 1) * D],
                zw_d[D + dc * 128 : D + (dc + 1) * 128, :],
            )
        zsb = pool.tile([128, 4 * D], BF16, tag="zsb")
        nc.vector.tensor_add(zsb[:], zs0[:], zs1[:])
        nc.sync.dma_start(zws_d[:], zsb[:])

    nc.compile()
    return nc


def _build_main(g1s, g2, ck):
    """Per-core main program (zero biases, shared g1 gate)."""
    nc = bacc.Bacc("TRN2", target_bir_lowering=False, debug=False)

    x_d = nc.dram_tensor("x", [NT, D], F32, kind="ExternalInput").ap()
    wphiT_d = nc.dram_tensor("wphiT", [DC, 128, K * D], BF16,
                             kind="ExternalInput").ap()
    w2T_d = nc.dram_tensor("w2T", [P, 128, 4 * D], BF16,
                           kind="ExternalInput").ap()
    zws_d = nc.dram_tensor("zws", [P, 128, 4 * D], BF16,
                           kind="ExternalInput").ap()
    out_d = nc.dram_tensor("out", [NT, P, D], F32, kind="ExternalOutput").ap()
    out_r = out_d.rearrange("(a b) p e -> b a p e", b=128)

    HT = NT // 2  # tokens per half (256)

    with tile.TileContext(nc) as tc, contextlib.ExitStack() as ctx:
        const_pool = ctx.enter_context(tc.tile_pool(name="consts", bufs=1))
        id32 = const_pool.tile([128, 128], F32)
        masks.make_identity(nc, id32[:])

        persist = ctx.enter_context(tc.tile_pool(name="persist", bufs=1))
        xE = persist.tile([128, DC * NT], F32, tag="xE")      # exp(x^T)
        xTb = persist.tile([128, DC * NT], BF16, tag="xTb")   # x^T in bf16
        ez = persist.tile([128, DC * NT], F32, tag="ez")      # exp(g1 x_proj^T)
        z0T = persist.tile([128, DC * NT], BF16, tag="z0T")
        zsb_pool = ctx.enter_context(tc.tile_pool(name="zsbp", bufs=P))
        zsb_pool_tiles = []

        with contextlib.ExitStack() as s1:
            ps_tr = s1.enter_context(
                tc.tile_pool(name="ps_tr", bufs=2, space="PSUM"))
            ps_mm1 = s1.enter_context(
                tc.tile_pool(name="ps_mm1", bufs=4, space="PSUM"))
            stage1 = s1.enter_context(tc.tile_pool(name="stage1", bufs=1))
            phi = [
                stage1.tile([128, DC * NT], BF16, tag=f"phi{k}", name=f"phi{k}")
                for k in range(K)
            ]

            # ---------------- stage A: load x, transpose, exp, phi ------------
            xa_pool = s1.enter_context(tc.tile_pool(name="xa", bufs=4))
            xn = []
            for j in range(NC):
                t = xa_pool.tile([128, D], F32, tag="xn", name=f"xn{j}")
                nc.sync.dma_start(t[:], x_d[j * 128 : (j + 1) * 128, :])
                xn.append(t)
            for dc in range(DC):
                pt = ps_tr.tile([128, NT], F32, tag="tr", name=f"xtp{dc}")
                for j in range(NC):
                    nc.tensor.transpose(
                        pt[:, j * 128 : (j + 1) * 128],
                        xn[j][:, dc * 128 : (dc + 1) * 128],
                        id32[:],
                    )
                nc.scalar.activation(xE[:, dc * NT : (dc + 1) * NT], pt[:], AF.Exp)
                nc.vector.tensor_copy(xTb[:, dc * NT : (dc + 1) * NT], pt[:])

            for k in range(K):
                nc.scalar.activation(
                    phi[k][:], xE[:], AF.Ln, bias=1.0, scale=float(ck[k])
                )

            # z_weight loads early (petal psB operands)
            for p in range(P):
                zsb = zsb_pool.tile([128, 4 * D], BF16, tag="zsb", name=f"zsb{p}")
                nc.sync.dma_start(zsb[:], zws_d[p])
                zsb_pool_tiles.append(zsb)

            # -------- stage B: mm1 (x_proj^T per e-chunk), token halves -------
            wphi_pool = s1.enter_context(tc.tile_pool(name="wphi", bufs=4))
            wsbs = []
            for ec in range(DC):
                wsb = wphi_pool.tile([128, K * D], BF16, tag="wsb", name=f"wsb{ec}")
                nc.sync.dma_start(wsb[:], wphiT_d[ec])
                wsbs.append(wsb)
            for half in range(2):
                hoff = half * HT
                for ec in range(DC):
                    xp_ps = ps_mm1.tile([128, HT], F32, tag="mm1",
                                        name=f"xp_ps{half}_{ec}")
                    first = True
                    for k in range(K):
                        for dc in range(DC):
                            off = (k // 4) * 2048 + (k % 4) * 512 + dc * 128
                            last = (k == K - 1) and (dc == DC - 1)
                            nc.tensor.matmul(
                                xp_ps[:],
                                wsbs[ec][:, off : off + 128],
                                phi[k][:, dc * NT + hoff : dc * NT + hoff + HT],
                                start=first,
                                stop=last,
                            )
                            first = False
                    # exp(g1 * x_proj^T) straight out of PSUM
                    nc.scalar.activation(
                        ez[:, ec * NT + hoff : ec * NT + hoff + HT],
                        xp_ps[:], AF.Exp, scale=float(g1s),
                    )
                # z0 for this token half (strided over the 4 e-chunks)
                ez_h = ez[:].rearrange("q (c n) -> q c n", c=DC)[:, :, hoff:hoff + HT]
                z0_h = z0T[:].rearrange("q (c n) -> q c n", c=DC)[:, :, hoff:hoff + HT]
                nc.scalar.activation(z0_h, ez_h, AF.Ln, bias=1.0)

        # ---------------- stage D: petals, half-major ----------------
        ps_pet = ctx.enter_context(tc.tile_pool(name="ps_pet", bufs=4,
                                                space="PSUM"))
        pet2_pool = ctx.enter_context(tc.tile_pool(name="pet2", bufs=P))
        zf_pool = ctx.enter_context(tc.tile_pool(name="zf", bufs=3))

        w2sbs = []
        for p in range(P):
            w2sb = pet2_pool.tile([128, 4 * D], BF16, tag="w2sb", name=f"w2sb{p}")
            nc.sync.dma_start(w2sb[:], w2T_d[p])
            w2sbs.append(w2sb)

        for half in range(2):
            hoff = half * HT
            for p in range(P):
                g2p = float(g2[p])
                psA = ps_pet.tile([128, 1024], F32, tag="pet",
                                  name=f"psA{p}_{half}")
                for js in range(2):
                    noff = hoff + js * 128
                    for dc in range(DC):
                        nc.tensor.matmul(
                            psA[:, js * D : (js + 1) * D],
                            z0T[:, dc * NT + noff : dc * NT + noff + 128],
                            w2sbs[p][:, dc * D : (dc + 1) * D],
                            start=(dc == 0),
                            stop=(dc == DC - 1),
                        )
                # z_final = ln(1 + (1 + e^{g2 u}) e^r)  [u = z0@w2T, r = x@zws]
                # == softplus(softplus(g2 u) + r); one ACT pass fewer.
                t1 = zf_pool.tile([128, 1024], F32, tag="t1", name=f"t1_{p}{half}")
                nc.scalar.activation(t1[:], psA[:], AF.Exp, scale=g2p)

                psB = ps_pet.tile([128, 1024], F32, tag="pet",
                                  name=f"psB{p}_{half}")
                for js in range(2):
                    noff = hoff + js * 128
                    for dc in range(DC):
                        nc.tensor.matmul(
                            psB[:, js * D : (js + 1) * D],
                            xTb[:, dc * NT + noff : dc * NT + noff + 128],
                            zsb_pool_tiles[p][:, dc * D : (dc + 1) * D],
                            start=(dc == 0),
                            stop=(dc == DC - 1),
                        )
                t2 = zf_pool.tile([128, 1024], F32, tag="t2", name=f"t2_{p}{half}")
                nc.scalar.activation(t2[:], psB[:], AF.Exp)
                nc.vector.tensor_mul(t1[:], t1[:], t2[:])
                nc.vector.tensor_add(t1[:], t1[:], t2[:])
                zf = zf_pool.tile([128, 1024], F32, tag="zfo", name=f"zf{p}{half}")
                nc.scalar.activation(zf[:], t1[:], AF.Ln, bias=1.0)

                zf_r = zf[:].rearrange("n (js e) -> n js e", js=2)
                nc.sync.dma_start(out_r[:, half * 2 : (half + 1) * 2, p, :], zf_r)

    nc.compile()
    return nc


def _prep_scalars(inputs):
    gate_raw = np.asarray(inputs["gate_raw"], dtype=np.float32)
    gate_raw2 = np.asarray(inputs["gate_raw2"], dtype=np.float32)
    g1 = 1.0 / (1.0 + np.exp(-gate_raw.astype(np.float64)))
    g2 = 1.0 / (1.0 + np.exp(-gate_raw2.astype(np.float64)))
    shifts = np.linspace(-1.0, 1.0, K, dtype=np.float32)
    ck = np.exp(shifts.astype(np.float64))

    if not bool(np.all(gate_raw == gate_raw[0])):
        raise NotImplementedError("per-petal gate_raw values")
    for name in ("phi_bias", "bias2", "output_bias"):
        if bool(np.any(np.asarray(inputs[name]))):
            raise NotImplementedError(f"nonzero {name} not supported")
    return g1, g2, ck


def _get_programs(inputs):
    g1, g2, ck = _prep_scalars(inputs)
    key = (tuple(np.float32(g1)), tuple(np.float32(g2)), tuple(np.float32(ck)))
    if key not in _CACHE:
        _CACHE[key] = (_build_prep(), _build_main(g1[0], g2, ck))
    return _CACHE[key]


def kernel(**inputs):
    nc_prep, nc_main = _get_programs(inputs)

    x = np.ascontiguousarray(np.asarray(inputs["x"], dtype=np.float32))
    orig_shape = x.shape
    x_flat = x.reshape(-1, D)
    assert x_flat.shape[0] == N_CORES * NT

    phr = np.asarray(inputs["phi_raw"], dtype=np.float32)
    w2 = np.asarray(inputs["raw_weight2"], dtype=np.float32)
    zw = np.asarray(inputs["z_weight"], dtype=np.float32)

    # ---- dispatch 1: parameter prep, work sharded across cores ----
    prep_maps = []
    for c in range(N_CORES):
        ec, kh = c // 2, c % 2
        rows = phr[ec * 128 : (ec + 1) * 128].reshape(128, D, K)
        phr_part = np.ascontiguousarray(
            rows[:, :, kh * 4 : (kh + 1) * 4].transpose(0, 2, 1).reshape(128, 4 * D)
        )
        w2_part = np.ascontiguousarray(
            w2[c].reshape(4, 128, D).transpose(1, 0, 2).reshape(128, 4 * D)
        )
        prep_maps.append(
            {
                "phr_part": phr_part,
                "w2_part": w2_part,
                "zw_part": np.ascontiguousarray(zw[c]),
            }
        )
    res1 = run_bass_kernel_spmd(nc_prep, prep_maps, core_ids=list(range(N_CORES)),
                                **_RUN_KWARGS)

    wphiT = np.empty((DC, 128, K * D), dtype=NPBF16)
    w2T = np.empty((P, 128, 4 * D), dtype=NPBF16)
    zws = np.empty((P, 128, 4 * D), dtype=NPBF16)
    for c in range(N_CORES):
        r = res1.results[c]
        wphiT[c // 2, :, (c % 2) * 2048 : (c % 2 + 1) * 2048] = r["wphiT_part"]
        w2T[c] = r["w2T_part"]
        zws[c] = r["zws_part"]

    # ---- dispatch 2: main, data parallel over tokens ----
    main_maps = []
    for c in range(N_CORES):
        main_maps.append(
            {
                "x": np.ascontiguousarray(x_flat[c * NT : (c + 1) * NT]),
                "wphiT": wphiT,
                "w2T": w2T,
                "zws": zws,
            }
        )
    res2 = run_bass_kernel_spmd(nc_main, main_maps, core_ids=list(range(N_CORES)),
                                **_RUN_KWARGS)

    out = np.concatenate([res2.results[c]["out"] for c in range(N_CORES)], axis=0)
    kernel.last_results = (res1, res2)
    return out.reshape(tuple(orig_shape[:-1]) + (P, D))


kernel.last_results = None
